# revision 4
# baseline (speedup 1.0000x reference)
"""Bagging autoencoder ensemble kernel for 8 Trainium2 NeuronCores.

Strategy
--------
Batch-parallel: each core gets B/8 = 512 batch rows and computes all E=100
estimators on them. Host-side weight prep removes the gather entirely
(x[:, idx[e]] @ We0[e]  ==  x @ scatter_add(We0[e], idx[e])), packs 8
estimators per matmul via concatenated / block-diagonal weights so the tiny
per-estimator layers run as dense 128-wide matmuls, and folds the final-layer
bias in via an augmented constant-one feature. All matmuls run as float32r
(FP22 multiply, fp32 accumulate) at full PE rate.

Per-core dataflow (activations kept as [feature_stack, batch] in SBUF):
  h0[128,512] = W0s_g.T @ xT          (K=256 over 2 tiles, 8 estimators)
  h1[64,512]  = blockdiag(We1).T @ h0 (+bias, relu)
  z [64,512]  = blockdiag(Wl).T @ h1  (+bias, relu)
  d0[128,512] = blockdiag(Wd0).T @ z  (+bias)
  d1[66,512]  = blockdiag-pair(Wd1aug).T @ d0 (+bias, relu; 33rd row == 1)
  o [128,512] = d1_bsub.T @ Wo_aug    (per 128-batch subtile, 2 est x 256 out)
  sigmoid -> staged [128, 2048] -> one 1 MB DMA per estimator pair
"""

import os
import sys

import numpy as np

for _p in ("/opt/trn_rl_repo", "/root/.axon_site/_ro/trn_rl_repo"):
    if os.path.isdir(_p) and _p not in sys.path:
        sys.path.append(_p)

import concourse.bass as bass
import concourse.mybir as mybir
import concourse.tile as tile
from concourse.bass_utils import run_bass_kernel_spmd

E, B, D, F, H, L = 100, 4096, 256, 32, 16, 8
N_CORES = 8
BC = B // N_CORES          # batch rows per core
G = 13                     # estimator groups of 8 (E padded 100 -> 104)
GE = 8                     # estimators per group
NPAIR = G * GE // 2        # 52 pairs incl. 2 padding pairs
NPAIR_REAL = E // 2        # 50
MA = 33                    # augmented d1 features per estimator (32 + ones row)
F32 = mybir.dt.float32
F32R = mybir.dt.float32r


def _host_prep(x, idx, We0, be0, We1, be1, Wl, bl, Wd0, bd0, Wd1, bd1, Wo, bo):
    f32 = np.float32
    x = np.ascontiguousarray(np.asarray(x, f32))
    idx = np.asarray(idx).astype(np.int64)
    We0, be0 = np.asarray(We0, f32), np.asarray(be0, f32)
    We1, be1 = np.asarray(We1, f32), np.asarray(be1, f32)
    Wl, bl = np.asarray(Wl, f32), np.asarray(bl, f32)
    Wd0, bd0 = np.asarray(Wd0, f32), np.asarray(bd0, f32)
    Wd1, bd1 = np.asarray(Wd1, f32), np.asarray(bd1, f32)
    Wo, bo = np.asarray(Wo, f32), np.asarray(bo, f32)

    # Fold the per-estimator feature gather into a scattered first-layer weight.
    W0s = np.zeros((E, D, H), f32)
    for e in range(E):
        np.add.at(W0s[e], idx[e], We0[e])

    w0s = np.zeros((128, G * 2 * 128), f32)
    b0g = np.zeros((128, G), f32)
    wb1 = np.zeros((128, G * 64), f32)
    b1g = np.zeros((64, G), f32)
    wbl = np.zeros((64, G * 64), f32)
    blg = np.zeros((64, G), f32)
    wd0 = np.zeros((64, G * 128), f32)
    bd0g = np.zeros((128, G), f32)
    for g in range(G):
        for j in range(GE):
            e = g * GE + j
            if e >= E:
                continue
            for t in range(2):
                w0s[:, (g * 2 + t) * 128 + j * H:(g * 2 + t) * 128 + (j + 1) * H] = \
                    W0s[e, t * 128:(t + 1) * 128, :]
            b0g[j * H:(j + 1) * H, g] = be0[e]
            wb1[j * H:(j + 1) * H, g * 64 + j * L:g * 64 + (j + 1) * L] = We1[e]
            b1g[j * L:(j + 1) * L, g] = be1[e]
            wbl[j * L:(j + 1) * L, g * 64 + j * L:g * 64 + (j + 1) * L] = Wl[e]
            blg[j * L:(j + 1) * L, g] = bl[e]
            wd0[j * L:(j + 1) * L, g * 128 + j * H:g * 128 + (j + 1) * H] = Wd0[e]
            bd0g[j * H:(j + 1) * H, g] = bd0[e]

    wd1 = np.zeros((128, NPAIR * 2 * MA), f32)
    bd1a = np.zeros((2 * MA, NPAIR), f32)
    for p in range(NPAIR):
        g, j0 = p // 4, (p % 4) * 2
        for c in range(2):
            j = j0 + c
            e = g * GE + j
            if e >= E:
                continue
            wd1[j * H:(j + 1) * H, p * 2 * MA + c * MA:p * 2 * MA + c * MA + F] = Wd1[e]
            bd1a[c * MA:c * MA + F, p] = bd1[e]
            bd1a[c * MA + F, p] = 1.0  # augmented constant-one feature (relu(0+1)=1)

    wo = np.zeros((NPAIR_REAL, 2 * MA, 2 * D), f32)
    for p in range(NPAIR_REAL):
        for c in range(2):
            e = 2 * p + c
            wo[p, c * MA:c * MA + F, c * D:(c + 1) * D] = Wo[e]
            wo[p, c * MA + F, c * D:(c + 1) * D] = bo[e]

    # per-core transposed x slice: [2, 128, BC], d = t*128 + r
    xts = [np.ascontiguousarray(x[c * BC:(c + 1) * BC, :].T.reshape(2, 128, BC))
           for c in range(N_CORES)]

    shared = dict(w0s=w0s, b0g=b0g, wb1=wb1, b1g=b1g, wbl=wbl, blg=blg,
                  wd0=wd0, bd0g=bd0g, wd1=wd1, bd1a=bd1a, wo=wo)
    return shared, xts


def _legalize_waits(nc, max_waits=1):
    """This neuronxcc encodes a single sem-wait slot per instruction; hoist
    overflow waits onto same-engine NoOps placed immediately before."""
    ctr = 0
    for f in nc.m.functions:
        for bb in f.blocks:
            out = []
            for inst in bb.instructions:
                si = inst.sync_info
                if si is not None and si.on_wait and len(si.on_wait) > max_waits:
                    waits = list(si.on_wait)
                    extra, keep = waits[:-max_waits], waits[-max_waits:]
                    for j in range(0, len(extra), max_waits):
                        nop = mybir.InstNoOp(name=f"I-waitsplit-{ctr}")
                        ctr += 1
                        nop.engine = inst.engine
                        nop.sync_info = mybir.SyncInfo(
                            on_wait=extra[j:j + max_waits], on_update=[])
                        out.append(nop)
                    inst.sync_info = mybir.SyncInfo(
                        on_wait=keep, on_update=list(si.on_update or []))
                out.append(inst)
            bb.instructions[:] = out


def _build_nc(legalize=True):
    nc = bass.Bass("TRN2", target_bir_lowering=False, debug=False,
                   num_devices=N_CORES)
    xt_d = nc.declare_dram_parameter("xt", [2, 128, BC], F32, isOutput=False)
    w0s_d = nc.declare_dram_parameter("w0s", [128, G * 2 * 128], F32, isOutput=False)
    b0g_d = nc.declare_dram_parameter("b0g", [128, G], F32, isOutput=False)
    wb1_d = nc.declare_dram_parameter("wb1", [128, G * 64], F32, isOutput=False)
    b1g_d = nc.declare_dram_parameter("b1g", [64, G], F32, isOutput=False)
    wbl_d = nc.declare_dram_parameter("wbl", [64, G * 64], F32, isOutput=False)
    blg_d = nc.declare_dram_parameter("blg", [64, G], F32, isOutput=False)
    wd0_d = nc.declare_dram_parameter("wd0", [64, G * 128], F32, isOutput=False)
    bd0g_d = nc.declare_dram_parameter("bd0g", [128, G], F32, isOutput=False)
    wd1_d = nc.declare_dram_parameter("wd1", [128, NPAIR * 2 * MA], F32, isOutput=False)
    bd1a_d = nc.declare_dram_parameter("bd1a", [2 * MA, NPAIR], F32, isOutput=False)
    wo_d = nc.declare_dram_parameter("wo", [NPAIR_REAL, 2 * MA, 2 * D], F32, isOutput=False)
    out_d = nc.declare_dram_parameter("out", [E, BC, D], F32, isOutput=True)

    ADD = mybir.AluOpType.add
    MAX = mybir.AluOpType.max
    SIG = mybir.ActivationFunctionType.Sigmoid

    with tile.TileContext(nc) as tc:
        with (
            tc.tile_pool(name="const", bufs=1) as cp,
            tc.tile_pool(name="wop", bufs=4) as wop,
            tc.tile_pool(name="mids", bufs=2) as mids,
            tc.tile_pool(name="d1p", bufs=3) as d1p,
            tc.tile_pool(name="stage", bufs=2) as stp,
            tc.tile_pool(name="ps_mid", bufs=2, space="PSUM") as ps_mid,
            tc.tile_pool(name="ps_d1", bufs=2, space="PSUM") as ps_d1,
            tc.tile_pool(name="ps_o", bufs=4, space="PSUM") as ps_o,
        ):
            xt0 = cp.tile([128, BC], F32R, tag="xt0")
            nc.sync.dma_start(out=xt0[:], in_=xt_d[0].bitcast(F32R))
            xt1 = cp.tile([128, BC], F32R, tag="xt1")
            nc.sync.dma_start(out=xt1[:], in_=xt_d[1].bitcast(F32R))
            w0s_t = cp.tile([128, G * 2 * 128], F32R, tag="w0s")
            nc.sync.dma_start(out=w0s_t[:], in_=w0s_d[:, :].bitcast(F32R))
            wb1_t = cp.tile([128, G * 64], F32R, tag="wb1")
            nc.sync.dma_start(out=wb1_t[:], in_=wb1_d[:, :].bitcast(F32R))
            wbl_t = cp.tile([64, G * 64], F32R, tag="wbl")
            nc.sync.dma_start(out=wbl_t[:], in_=wbl_d[:, :].bitcast(F32R))
            wd0_t = cp.tile([64, G * 128], F32R, tag="wd0")
            nc.sync.dma_start(out=wd0_t[:], in_=wd0_d[:, :].bitcast(F32R))
            wd1_t = cp.tile([128, NPAIR * 2 * MA], F32R, tag="wd1")
            nc.sync.dma_start(out=wd1_t[:], in_=wd1_d[:, :].bitcast(F32R))
            b0_t = cp.tile([128, G], F32, tag="b0")
            nc.sync.dma_start(out=b0_t[:], in_=b0g_d[:, :])
            b1_t = cp.tile([64, G], F32, tag="b1")
            nc.sync.dma_start(out=b1_t[:], in_=b1g_d[:, :])
            bl_t = cp.tile([64, G], F32, tag="bl")
            nc.sync.dma_start(out=bl_t[:], in_=blg_d[:, :])
            bd0_t = cp.tile([128, G], F32, tag="bd0")
            nc.sync.dma_start(out=bd0_t[:], in_=bd0g_d[:, :])
            bd1_t = cp.tile([2 * MA, NPAIR], F32, tag="bd1")
            nc.sync.dma_start(out=bd1_t[:], in_=bd1a_d[:, :])

            for g in range(G):
                ps = ps_mid.tile([128, BC], F32, tag="psm")
                nc.tensor.matmul(ps[:], w0s_t[:, (2 * g) * 128:(2 * g + 1) * 128],
                                 xt0[:], start=True, stop=False)
                nc.tensor.matmul(ps[:], w0s_t[:, (2 * g + 1) * 128:(2 * g + 2) * 128],
                                 xt1[:], start=False, stop=True)
                h0 = mids.tile([128, BC], F32R, tag="h0")
                nc.vector.tensor_scalar(h0[:], ps[:], b0_t[:, g:g + 1], None, ADD)

                ps2 = ps_mid.tile([64, BC], F32, tag="psm")
                nc.tensor.matmul(ps2[:], wb1_t[:, g * 64:(g + 1) * 64], h0[:],
                                 start=True, stop=True)
                h1 = mids.tile([64, BC], F32R, tag="h1")
                nc.vector.tensor_scalar(h1[:], ps2[:], b1_t[:, g:g + 1], 0.0, ADD, MAX)

                ps3 = ps_mid.tile([64, BC], F32, tag="psm")
                nc.tensor.matmul(ps3[:], wbl_t[:, g * 64:(g + 1) * 64], h1[:],
                                 start=True, stop=True)
                zt = mids.tile([64, BC], F32R, tag="zt")
                nc.vector.tensor_scalar(zt[:], ps3[:], bl_t[:, g:g + 1], 0.0, ADD, MAX)

                ps4 = ps_mid.tile([128, BC], F32, tag="psm")
                nc.tensor.matmul(ps4[:], wd0_t[:, g * 128:(g + 1) * 128], zt[:],
                                 start=True, stop=True)
                d0 = mids.tile([128, BC], F32R, tag="d0")
                nc.vector.tensor_scalar(d0[:], ps4[:], bd0_t[:, g:g + 1], None, ADD)

                for pl in range(4):
                    p = g * 4 + pl
                    if p >= NPAIR_REAL:
                        continue
                    wo_t = wop.tile([2 * MA, 2 * D], F32R, tag="wo")
                    nc.sync.dma_start(out=wo_t[:], in_=wo_d[p].bitcast(F32R))

                    psd = ps_d1.tile([2 * MA, BC], F32, tag="psd")
                    nc.tensor.matmul(psd[:], wd1_t[:, p * 2 * MA:(p + 1) * 2 * MA],
                                     d0[:], start=True, stop=True)
                    d1 = d1p.tile([2 * MA, BC], F32R, tag="d1")
                    nc.vector.tensor_scalar(d1[:], psd[:], bd1_t[:, p:p + 1], 0.0, ADD, MAX)

                    stage = stp.tile([128, 2 * 4 * D], F32, tag="stage")
                    st4 = stage[:].rearrange("q (e s d) -> q e s d", e=2, s=4, d=D)
                    for s in range(4):
                        pso = ps_o.tile([128, 2 * D], F32, tag="pso")
                        nc.tensor.matmul(pso[:], d1[:, s * 128:(s + 1) * 128],
                                         wo_t[:], start=True, stop=True)
                        nc.scalar.activation(st4[:, :, s, :],
                                             pso[:].rearrange("q (e d) -> q e d", e=2),
                                             SIG)
                    out_view = out_d.ap()[2 * p:2 * p + 2].rearrange(
                        "e (s q) d -> q e s d", s=4, q=128)
                    nc.sync.dma_start(out=out_view, in_=st4)

    if legalize:
        _legalize_waits(nc)
    return nc


_NC_CACHE = []


def kernel(x, idx, We0, be0, We1, be1, Wl, bl, Wd0, bd0, Wd1, bd1, Wo, bo,
           _trace=False, _trace_cores=None):
    shared, xts = _host_prep(x, idx, We0, be0, We1, be1, Wl, bl,
                             Wd0, bd0, Wd1, bd1, Wo, bo)
    if not _NC_CACHE:
        _NC_CACHE.append(_build_nc())
    nc = _NC_CACHE[0]
    in_maps = [dict(shared, xt=xts[c]) for c in range(N_CORES)]
    res = run_bass_kernel_spmd(nc, in_maps, list(range(N_CORES)),
                               trace=_trace, trace_cores=_trace_cores)
    out = np.concatenate([res.results[c]["out"] for c in range(N_CORES)], axis=1)
    if _trace:
        return out, res
    return out
